# revision 1
# baseline (speedup 1.0000x reference)
"""Trainium2 Bass kernel for capsule routing (nn_Capsule).

Reference computation:
    u_hat = einsum('bic,ce->bie', u_vecs, W).reshape(B, I, N, D).transpose(0,2,1,3)
    b = 0
    for r in range(3):
        c = softmax(b, axis=1)                      # over capsules n
        out = squash(einsum('bni,bnid->bnd', c, u_hat))
        if r < 2: b = einsum('bnd,bnid->bni', out, u_hat)
    return out    # (B, N, D)

Key algebraic restructuring (u_hat is never materialized; it is 32 MiB per
core and every use of it factors through u_vecs and W):
    round 0:  c uniform = 1/N  ->  out0 = squash((1/N) * (sum_i u[b,i,:]) @ W)
    logits[b,i,n] = sum_c u[b,i,c] * V[b,c,n],   V[b,c,n] = sum_d W[c,(n,d)] o[b,n,d]
    T[b,n,c]     = sum_i softmax(logits)[b,i,n] * u[b,i,c]
    pre[b,n,d]   = sum_c T[b,n,c] * W[c,(n,d)]   -> out = squash(pre)

Implementation notes:
  - T/pre/squash matmuls use float32r views (single-pass FP22 PE matmul;
    plain fp32 is split into two passes by the compiler, doubling PE time);
    the logits and V matmuls run in bf16, whose full-128-column stationaries
    get the compiler's fast-weight-load (4x LDWEIGHTS) path,
  - squash's rsqrt is a bit-trick seed + 2 fused Newton steps on the Vector
    engine, so the Scalar engine's activation table stays pinned on exp
    (softmax) and never pays the ~2.7us table-set switch,
  - softmax is over the free dim in an (i, n) layout; sums over the
    capsule dim d use ones-vector matmuls on the PE,
  - per-batch logits->softmax->T chains are emitted per b so PE work of
    batch b+1 overlaps the DVE/ACT softmax of batch b.

Sharding: data-parallel over batch, 4 batches per core x 8 cores, W replicated.
"""

import numpy as np
from contextlib import ExitStack

import concourse.bass as bass
import concourse.bacc as bacc
import concourse.tile as tile
from concourse import mybir
from concourse.bass_utils import run_bass_kernel_spmd
from concourse.masks import make_identity

B, I, C = 32, 1024, 256
N, D = 32, 64
ND = N * D
ROUTINGS = 3
EPS = 1e-7
NCORES = 8
BL = B // NCORES  # batches per core
IC = I // 128     # i chunks of 128
CK = C // 128     # c chunks of 128
NB = N * BL       # 128 = (n, b) composite
F32 = mybir.dt.float32
F32R = mybir.dt.float32r
U32 = mybir.dt.uint32
BF16 = mybir.dt.bfloat16
MULT = mybir.AluOpType.mult
AF = mybir.ActivationFunctionType
RSQRT_MAGIC = 0x5F3759DF


def _r(ap):
    """View an fp32 AP as float32r: single-pass (FP22) PE matmul instead of
    the 2-pass fp32 split walrus emits otherwise."""
    return ap.bitcast(F32R)


def _capsule_body(ctx: ExitStack, tc: tile.TileContext, out_ap, u_ap, w_ap):
    nc = tc.nc

    # f32r out-views (PE single-pass matmul inputs) trip the low-precision
    # accumulation guard; the rounding loss (22-bit mantissa) is intentional.
    ctx.enter_context(nc.allow_low_precision(reason="fp32r single-pass matmuls"))

    const = ctx.enter_context(tc.tile_pool(name="const", bufs=1))
    persist = ctx.enter_context(tc.tile_pool(name="persist", bufs=1))
    work = ctx.enter_context(tc.tile_pool(name="work", bufs=4))

    # ---- constants ----
    ident = const.tile([128, 128], F32)
    make_identity(nc, ident[:])
    ones_f = const.tile([128, 2], F32)
    nc.gpsimd.memset(ones_f[:], 1.0)
    ones_col = const.tile([128, 1], F32)
    nc.vector.tensor_copy(out=_r(ones_col[:]), in_=ones_f[:, 0:1])
    ones_row = const.tile([1, 128], F32)
    nc.vector.tensor_copy(out=_r(ones_row[:]), in_=ones_f[0:1, 0:1].to_broadcast([1, 128]))
    magic = const.tile([1, NB], U32)
    nc.gpsimd.memset(magic[:], RSQRT_MAGIC)

    # ---- persistent SBUF tensors ----
    w_sb = persist.tile([128, CK, ND], F32)       # [q, ck, (n,d)]
    wt_sb = persist.tile([64, N, C], BF16)        # [d, n, c] (bf16: V stationary, FWL)
    u_sb = persist.tile([128, BL, IC, C], F32)    # [p, b, ic, c]
    ut_sb = persist.tile([128, BL, CK, I], BF16)  # [q, b, ck, i] (bf16: lg stationary, FWL)
    st_sb = persist.tile([128, CK, BL], F32)      # [q, ck, b]  (column sums of u)

    # ---- load inputs ----
    for ck in range(CK):
        nc.sync.dma_start(out=_r(w_sb[:, ck, :]), in_=_r(w_ap[ck * 128:(ck + 1) * 128, :]))
    for b in range(BL):
        for ic in range(IC):
            nc.sync.dma_start(
                out=_r(u_sb[:, b, ic, :]),
                in_=_r(u_ap[b, ic * 128:(ic + 1) * 128, :]),
            )

    # ---- setup transposes (PE) ----
    with tc.tile_pool(name="ps_setup", bufs=4, space="PSUM") as ps_setup, \
            nc.named_scope("setup"):
        # u blocks: ut[q, b, ck, ic*128:+128] = u[b, i-chunk, c-chunk].T
        for b in range(BL):
            for ck in range(CK):
                for ic in range(IC):
                    ut_ps = ps_setup.tile([128, 128], F32, tag="ut")
                    nc.tensor.transpose(
                        ut_ps[:], u_sb[:, b, ic, ck * 128:(ck + 1) * 128], ident[:]
                    )
                    if (ic + ck) % 2 == 0:
                        nc.vector.tensor_copy(
                            out=ut_sb[:, b, ck, ic * 128:(ic + 1) * 128], in_=ut_ps[:]
                        )
                    else:
                        nc.scalar.copy(
                            out=ut_sb[:, b, ck, ic * 128:(ic + 1) * 128], in_=ut_ps[:]
                        )
        # column sums of u: st[q, ck, b] = sum_i u[b, i, ck-chunk]
        for b in range(BL):
            for ck in range(CK):
                nc.vector.reduce_sum(
                    out=_r(st_sb[:, ck, b:b + 1]),
                    in_=ut_sb[:, b, ck, :],
                    axis=mybir.AxisListType.X,
                )
        # W blocks: wt[d, n, ck*128:+128] = W[ck-chunk, n-block].T
        for ck in range(CK):
            for n in range(N):
                wt_ps = ps_setup.tile([64, 128], F32, tag="wt")
                nc.tensor.transpose(
                    wt_ps[:], w_sb[:, ck, n * 64:(n + 1) * 64], ident[:]
                )
                if n % 2 == 0:
                    nc.vector.tensor_copy(
                        out=wt_sb[0:64, n, ck * 128:(ck + 1) * 128], in_=wt_ps[:]
                    )
                else:
                    nc.scalar.copy(
                        out=wt_sb[0:64, n, ck * 128:(ck + 1) * 128], in_=wt_ps[:]
                    )

    ps = ctx.enter_context(tc.tile_pool(name="ps_main", bufs=1, space="PSUM"))
    ps_pre = ctx.enter_context(tc.tile_pool(name="ps_pre", bufs=1, space="PSUM"))
    ps_t = ctx.enter_context(tc.tile_pool(name="ps_t", bufs=2, space="PSUM"))

    o_sb = None
    for r in range(ROUTINGS):
        if r > 0:
            # V[b][c, n] = sum_d W[c,(n,d)] o[b,n,d]
            with nc.named_scope(f"r{r}_v"):
                v_ps = ps.tile([128, CK, N, BL], F32, tag="v")
                for ck in range(CK):
                    for n in range(N):
                        nc.tensor.matmul(
                            out=v_ps[:, ck, n, :],
                            lhsT=wt_sb[0:64, n, ck * 128:(ck + 1) * 128],
                            rhs=o_sb[:, n * BL:(n + 1) * BL],
                            start=True,
                            stop=True,
                        )
                v_sb = work.tile([128, CK, N, BL], BF16, tag="v_sb")
                for ck in range(CK):
                    nc.scalar.copy(out=v_sb[:, ck], in_=v_ps[:, ck])

            # Per local batch: logits -> softmax -> T -> T^T, pipelined so b+1's
            # PE work overlaps b's DVE/ACT softmax.
            lg_ps = ps.tile([128, BL, IC, N], F32, tag="lg")
            tt_ps = ps.tile([128, CK, N, BL], F32, tag="tt")
            for b in range(BL):
                with nc.named_scope(f"r{r}_lg"):
                    for ic in range(IC):
                        for ck in range(CK):
                            nc.tensor.matmul(
                                out=lg_ps[:, b, ic, :],
                                lhsT=ut_sb[:, b, ck, ic * 128:(ic + 1) * 128],
                                rhs=v_sb[:, ck, :, b],
                                start=(ck == 0),
                                stop=(ck == CK - 1),
                            )
                # softmax over n (free dim; no max-subtraction, logits O(1))
                with nc.named_scope(f"r{r}_sm"):
                    e_sb = work.tile([128, IC, N], F32, tag="e")
                    nc.scalar.activation(out=e_sb[:], in_=lg_ps[:, b], func=AF.Exp)
                    s_sb = work.tile([128, IC], F32, tag="s")
                    nc.vector.reduce_sum(
                        out=s_sb[:], in_=e_sb[:], axis=mybir.AxisListType.X
                    )
                    sr_sb = work.tile([128, IC], F32, tag="sr")
                    nc.vector.reciprocal(out=sr_sb[:], in_=s_sb[:])
                    c_sb = work.tile([128, IC, N], F32, tag="c")
                    nc.vector.tensor_tensor(
                        _r(c_sb[:]),
                        e_sb[:],
                        sr_sb[:, :, None].to_broadcast([128, IC, N]),
                        MULT,
                    )
                # T[b][n, c] = sum_i c[i, n] u[b, i, c]
                with nc.named_scope(f"r{r}_t"):
                    t_ps = ps_t.tile([32, C], F32, tag="t")
                    for ic in range(IC):
                        nc.tensor.matmul(
                            out=t_ps[:],
                            lhsT=_r(c_sb[:, ic, :]),
                            rhs=_r(u_sb[:, b, ic, :]),
                            start=(ic == 0),
                            stop=(ic == IC - 1),
                        )
                    t_sb = work.tile([32, C], F32, tag="t_sb")
                    if b % 2 == 0:
                        nc.scalar.copy(out=t_sb[:], in_=t_ps[:])
                    else:
                        nc.vector.tensor_copy(out=t_sb[:], in_=t_ps[:])
                    for ck in range(CK):
                        nc.tensor.transpose(
                            tt_ps[:, ck, :, b],
                            t_sb[:, ck * 128:(ck + 1) * 128],
                            ident[0:32, 0:32],
                        )
            with nc.named_scope(f"r{r}_t"):
                tt_sb = work.tile([128, CK, N, BL], F32, tag="tt_sb")
                nc.vector.tensor_copy(out=_r(tt_sb[:]), in_=tt_ps[:])

        # ---------- pre[d, (n,b)] per-capsule: pre_n = W_n.T @ T_n ----------
        with nc.named_scope(f"r{r}_pre"):
            pre_ps = ps_pre.tile([64, N, BL], F32, tag="pre")
            for n in range(N):
                for ck in range(CK):
                    rhs = (
                        st_sb[:, ck, :] if r == 0 else tt_sb[:, ck, n, :]
                    )
                    nc.tensor.matmul(
                        out=pre_ps[:, n, :],
                        lhsT=_r(w_sb[:, ck, n * 64:(n + 1) * 64]),
                        rhs=_r(rhs),
                        start=(ck == 0),
                        stop=(ck == CK - 1),
                    )

        # ---------- squash over d (partition dim -> ones-matmul reductions;
        # rsqrt via bit-trick seed + 3 Newton steps, all DVE: keeps the ACT
        # table set pinned to exp for softmax) ----------
        with nc.named_scope(f"r{r}_sq"):
            pre_sb = work.tile([64, NB], F32, tag="pre_sb")
            nc.scalar.copy(out=pre_sb[:], in_=pre_ps[:].rearrange("d n b -> d (n b)"))
            sq_sb = work.tile([64, NB], F32, tag="sq")
            nc.vector.tensor_mul(_r(sq_sb[:]), pre_sb[:], pre_sb[:])
            ss_ps = ps.tile([1, NB], F32, tag="sqps")
            nc.tensor.matmul(
                out=ss_ps[:], lhsT=_r(ones_col[0:64, :]), rhs=_r(sq_sb[:]),
                start=True, stop=True,
            )
            # x = sum/N^2 + eps for r==0 (squash of pre/N), else sum + eps
            x_sb = work.tile([1, NB], F32, tag="x")
            nc.vector.tensor_scalar(
                out=x_sb[:], in0=ss_ps[:],
                scalar1=(1.0 / (N * N) if r == 0 else 1.0), scalar2=EPS,
                op0=MULT, op1=mybir.AluOpType.add,
            )
            # y0 = bitcast(0x5f3759df - (bitcast(x) >> 1))
            yb_sb = work.tile([1, NB], U32, tag="yb")
            nc.vector.tensor_scalar(
                out=yb_sb[:], in0=x_sb[:].bitcast(U32), scalar1=1, scalar2=None,
                op0=mybir.AluOpType.logical_shift_right,
            )
            y_sb = work.tile([1, NB], F32, tag="y")
            nc.vector.tensor_tensor(
                y_sb[:].bitcast(U32), magic[:], yb_sb[:],
                mybir.AluOpType.subtract,
            )
            # Newton: y <- y * (1.5 - 0.5 x y^2), twice (rsqrt rel err ~4e-6)
            for it in range(2):
                t1 = work.tile([1, NB], F32, tag="nt1")
                nc.vector.tensor_mul(t1[:], y_sb[:], y_sb[:])
                nc.vector.scalar_tensor_tensor(
                    out=t1[:], in0=t1[:], scalar=-0.5, in1=x_sb[:],
                    op0=MULT, op1=MULT,
                )
                y2 = work.tile([1, NB], F32, tag="y")
                nc.vector.scalar_tensor_tensor(
                    out=_r(y2[:]), in0=t1[:], scalar=1.5, in1=y_sb[:],
                    op0=mybir.AluOpType.add, op1=MULT,
                )
                y_sb = y2
            if r == 0:
                nc.vector.tensor_scalar_mul(_r(y_sb[:]), y_sb[:], 1.0 / N)
            rnb_ps = ps.tile([64, NB], F32, tag="sqps")
            nc.tensor.matmul(
                out=rnb_ps[:], lhsT=_r(ones_row[0:1, 0:64]), rhs=_r(y_sb[:]),
                start=True, stop=True,
            )
            if r < ROUTINGS - 1:
                o_sb = work.tile([64, NB], BF16, tag="o_bf")
                nc.vector.tensor_tensor(o_sb[:], pre_sb[:], rnb_ps[:], MULT)
            else:
                o_sb = work.tile([64, NB], F32, tag="o")
                nc.vector.tensor_tensor(_r(o_sb[:]), pre_sb[:], rnb_ps[:], MULT)

    # ---------- write out: out[b, n, d] = o[d, (n,b)] ----------
    with nc.named_scope("out"):
        ot_ps = ps.tile([128, 64], F32, tag="sqps")
        nc.tensor.transpose(ot_ps[:], o_sb[:], ident[0:64, 0:64])
        ot_sb = work.tile([128, 64], F32, tag="ot")
        nc.scalar.copy(out=ot_sb[:], in_=ot_ps[:])
        out_nbd = bass.AP(
            tensor=out_ap.tensor,
            offset=out_ap.offset,
            ap=[[D, N], [N * D, BL], [1, D]],
        )
        nc.sync.dma_start(out=out_nbd, in_=ot_sb[:])

def build_program():
    nc = bacc.Bacc("TRN2", target_bir_lowering=False, debug=False)
    u_ap = nc.dram_tensor("u", [BL, I, C], F32, kind="ExternalInput").ap()
    w_ap = nc.dram_tensor("w", [C, ND], F32, kind="ExternalInput").ap()
    out_ap = nc.dram_tensor("out", [BL, N, D], F32, kind="ExternalOutput").ap()
    with tile.TileContext(nc) as tc:
        with ExitStack() as ctx:
            _capsule_body(ctx, tc, out_ap, u_ap, w_ap)
    nc.compile()
    return nc


_NC = None


def kernel(u_vecs: np.ndarray, W: np.ndarray) -> np.ndarray:
    global _NC
    u = np.ascontiguousarray(np.asarray(u_vecs, dtype=np.float32))
    w = np.ascontiguousarray(np.asarray(W, dtype=np.float32))
    assert u.shape == (B, I, C) and w.shape == (C, ND)
    if _NC is None:
        _NC = build_program()
    in_maps = [
        {"u": u[i * BL:(i + 1) * BL], "w": w} for i in range(NCORES)
    ]
    res = run_bass_kernel_spmd(_NC, in_maps, list(range(NCORES)))
    return np.concatenate(
        [res.results[i]["out"] for i in range(NCORES)], axis=0
    )



# revision 16
# speedup vs baseline: 1.1977x; 1.1977x over previous
"""Trainium2 Bass kernel for capsule routing (nn_Capsule).

Reference computation:
    u_hat = einsum('bic,ce->bie', u_vecs, W).reshape(B, I, N, D).transpose(0,2,1,3)
    b = 0
    for r in range(3):
        c = softmax(b, axis=1)                      # over capsules n
        out = squash(einsum('bni,bnid->bnd', c, u_hat))
        if r < 2: b = einsum('bnd,bnid->bni', out, u_hat)
    return out    # (B, N, D)

Algebraic restructuring (u_hat never materialized; all uses factor through
u_vecs and W):
    round 0:  c uniform = 1/N  ->  out0 = squash((1/N) * W^T (sum_i u[b,i,:]))
    logits[b,i,n] = sum_c u[b,i,c] V[b,c,n],   V[b,c,n] = sum_d W[c,(n,d)] o[b,n,d]
    tt[b,c,n]    = sum_i softmax(logits)[b,i,n] * u[b,i,c]     (T transposed)
    pre[b,n,d]   = sum_c tt[b,c,n] W[c,(n,d)]   -> out = squash(pre)

PE-efficiency notes (LDWEIGHTS cost ~ stationary_columns/1.2 ns, halved by
fast-weight-load which triggers on 128-column non-fp32 stationaries):
  - every routing matmul uses a bf16 128-column stationary (FWL): u chunks for
    tt, u^T chunks for logits, W[:, 128-col] for pre, paired W^T for V,
  - tt is computed directly in [c, n] layout (u chunk stationary, softmax
    weights streamed) so the per-batch T transposes of the naive layout vanish,
  - pre packs capsule pairs into one [c,128] stationary: out quadrants
    [0:64, 0:4] / [64:128, 4:8] hold pre_n / pre_{n+1}; cross-quadrants are
    discarded.  W^T for V is built with the same pairing (one [c,128]
    transpose per capsule pair),
  - squash rsqrt = bit-trick seed + Newton on DVE, keeping the Scalar
    activation table pinned on exp,
  - input DMAs are 6 x 1MB split across the two HW-DGE rings (SP + ACT).

Sharding: data-parallel over batch, 4 batches per core x 8 cores, W replicated.
"""

import numpy as np
from contextlib import ExitStack

import concourse.bass as bass
import concourse.bacc as bacc
import concourse.tile as tile
from concourse import mybir
from concourse.bass_utils import run_bass_kernel_spmd
from concourse.masks import make_identity

B, I, C = 32, 1024, 256
N, D = 32, 64
ND = N * D
ROUTINGS = 3
EPS = 1e-7
NCORES = 8
BL = B // NCORES  # batches per core
IC = I // 128     # i chunks of 128
CK = C // 128     # c chunks of 128
NB = N * BL       # 128 = (n, b) composite
NP = N // 2       # capsule pairs
NDK = ND // 128   # 128-col chunks of the (n,d) axis == NP
F32 = mybir.dt.float32
F32R = mybir.dt.float32r
U32 = mybir.dt.uint32
BF16 = mybir.dt.bfloat16
MULT = mybir.AluOpType.mult
AF = mybir.ActivationFunctionType
RSQRT_MAGIC = 0x5F3759DF


def _r(ap):
    """View an fp32 AP as float32r: single-pass (FP22) PE matmul instead of
    the 2-pass fp32 split walrus emits otherwise."""
    return ap.bitcast(F32R)


def _capsule_body(ctx: ExitStack, tc: tile.TileContext, out_ap, u_ap, w_ap):
    nc = tc.nc

    ctx.enter_context(nc.allow_low_precision(reason="bf16/fp32r matmul path"))

    const = ctx.enter_context(tc.tile_pool(name="const", bufs=1))
    persist = ctx.enter_context(tc.tile_pool(name="persist", bufs=1))
    work = ctx.enter_context(tc.tile_pool(name="work", bufs=4))

    # ---- constants ----
    ident = const.tile([128, 128], F32)
    make_identity(nc, ident[:])
    ident_bf = const.tile([128, 128], BF16)
    make_identity(nc, ident_bf[:])
    ones_f = const.tile([128, 2], F32)
    nc.gpsimd.memset(ones_f[:], 1.0)
    ones_col = const.tile([128, 1], F32)
    nc.vector.tensor_copy(out=_r(ones_col[:]), in_=ones_f[:, 0:1])
    ones_row = const.tile([1, 128], F32)
    nc.vector.tensor_copy(out=_r(ones_row[:]), in_=ones_f[0:1, 0:1].to_broadcast([1, 128]))
    magic = const.tile([1, NB], U32)
    nc.gpsimd.memset(magic[:], RSQRT_MAGIC)

    # ---- persistent SBUF tensors ----
    w_sb = persist.tile([128, CK, ND], F32)       # [c, ck, (n,d)]
    w_bf = persist.tile([128, CK, ND], BF16)      # bf16 copy (pre stationaries)
    wt_ev = persist.tile([64, NP, C], BF16)       # [d, p, c] = W_2p^T (V stationaries)
    wt_od = persist.tile([64, NP, C], BF16)       # [d, p, c] = W_{2p+1}^T
    u_sb = persist.tile([128, BL, IC, C], F32)    # [i, b, ic, c]
    u_bf = persist.tile([128, BL, IC, C], BF16)   # bf16 copy (tt stationaries)
    ut_bf = persist.tile([128, BL, CK, I], BF16)  # [c, b, ck, i] (lg stationaries)
    st_sb = persist.tile([128, CK, BL], F32)      # [c, ck, b] column sums of u
    st_bf = persist.tile([128, CK, BL], BF16)

    # ---- input DMAs: 6 x 1MB, split across the SP and ACT HW-DGE rings ----
    # sync ring: u[b=0], u[b=1]; scalar ring: W, u[b=2], u[b=3]
    def dma_u(eng, b):
        src = bass.AP(
            tensor=u_ap.tensor,
            offset=u_ap.offset + b * I * C,
            ap=[[C, 128], [128 * C, IC], [1, C]],
        )
        eng.dma_start(out=_r(u_sb[:, b, :, :]), in_=_r(src))

    dma_u(nc.sync, 0)
    for ck in range(CK):
        nc.sync.dma_start(
            out=_r(w_sb[:, ck, :]), in_=_r(w_ap[ck * 128:(ck + 1) * 128, :])
        )
    dma_u(nc.sync, 1)
    dma_u(nc.sync, 2)
    dma_u(nc.sync, 3)

    # ---- setup: casts, transposes, column sums ----
    cast_engs = [nc.vector, nc.gpsimd, nc.vector, nc.gpsimd]

    def _copy(idx, out, in_):
        # PSUM -> SBUF evacuation: only ACT and DVE can read PSUM
        if idx % 2 == 0:
            nc.scalar.copy(out=out, in_=in_)
        else:
            nc.vector.tensor_copy(out=out, in_=in_)

    with tc.tile_pool(name="ps_su", bufs=3, space="PSUM") as ps_su, \
            tc.tile_pool(name="ps_sw", bufs=2, space="PSUM") as ps_sw, \
            nc.named_scope("setup"):
        # W: cast to bf16, then paired transposes wt[(d,d'), p, c]
        for ck in range(CK):
            nc.gpsimd.tensor_copy(out=w_bf[:, ck, :], in_=w_sb[:, ck, :])
        for b in range(BL):
            # cast u[b] to bf16
            cast_engs[b].tensor_copy(out=u_bf[:, b, :, :], in_=u_sb[:, b, :, :])
            # transpose 4-chunk groups: ut[c, b, ck, :]
            for ck in range(CK):
                for j in range(IC // 4):
                    ut_ps = ps_su.tile([128, 4, 128], BF16, tag="ut")
                    for t in range(4):
                        icx = 4 * j + t
                        nc.tensor.transpose(
                            ut_ps[:, t, :],
                            u_bf[:, b, icx, ck * 128:(ck + 1) * 128],
                            ident_bf[:],
                        )
                    _copy(
                        b * 4 + ck * 2 + j,
                        ut_bf[:, b, ck, j * 512:(j + 1) * 512],
                        ut_ps[:].rearrange("c t i -> c (t i)"),
                    )
            # interleave W transposes behind the first u batch
            if b == 0:
                for p in range(NP):
                    wt_ps = ps_sw.tile([128, CK, 128], BF16, tag="wt")
                    for ck in range(CK):
                        nc.tensor.transpose(
                            wt_ps[:, ck, :],
                            w_bf[:, ck, p * 128:(p + 1) * 128],
                            ident_bf[:],
                        )
                    _copy(
                        p,
                        wt_ev[:, p, :],
                        wt_ps[0:64, :, :].rearrange("d k c -> d (k c)"),
                    )
                    _copy(
                        p + 1,
                        wt_od[:, p, :],
                        wt_ps[64:128, :, :].rearrange("d k c -> d (k c)"),
                    )
            # column sums st[c, ck, b] = sum_i ut[c, b, ck, i]
            for ck in range(CK):
                nc.vector.reduce_sum(
                    out=_r(st_sb[:, ck, b:b + 1]),
                    in_=ut_bf[:, b, ck, :],
                    axis=mybir.AxisListType.X,
                )
        nc.vector.tensor_copy(out=st_bf[:], in_=st_sb[:])

    ps = ctx.enter_context(tc.tile_pool(name="ps_main", bufs=1, space="PSUM"))
    ps_lg = ctx.enter_context(tc.tile_pool(name="ps_lg", bufs=3, space="PSUM"))
    ps_tt = ctx.enter_context(tc.tile_pool(name="ps_tt", bufs=1, space="PSUM"))
    ps_pre = ctx.enter_context(tc.tile_pool(name="ps_pre", bufs=1, space="PSUM"))

    o_sb = None
    for r in range(ROUTINGS):
        tt_bf = None
        if r > 0:
            # V[b][c, n] = sum_d W[c,(n,d)] o[b,n,d]; stationary = paired W^T
            with nc.named_scope(f"r{r}_v"):
                v_ps = ps.tile([128, CK, N, BL], F32, tag="v")
                for ck in range(CK):
                    for n in range(N):
                        wt = wt_ev if n % 2 == 0 else wt_od
                        nc.tensor.matmul(
                            out=v_ps[:, ck, n, :],
                            lhsT=wt[:, n // 2, ck * 128:(ck + 1) * 128],
                            rhs=o_sb[:, n * BL:(n + 1) * BL],
                            start=True,
                            stop=True,
                        )
                v_bf = work.tile([128, CK, N, BL], BF16, tag="v_bf")
                nc.scalar.copy(out=v_bf[:], in_=v_ps[:])

            # logits[b][i, n] = sum_c ut[c, i] V[c, n]   (all b first: the
            # per-b softmax chains run behind the PE's logits stream)
            lg_tiles = []
            with nc.named_scope(f"r{r}_lg"):
                for b in range(BL):
                    lg_ps = ps_lg.tile([128, IC, N], F32, tag="lg")
                    lg_tiles.append(lg_ps)
                    for ic in range(IC):
                        for ck in range(CK):
                            nc.tensor.matmul(
                                out=lg_ps[:, ic, :],
                                lhsT=ut_bf[:, b, ck, ic * 128:(ic + 1) * 128],
                                rhs=v_bf[:, ck, :, b],
                                start=(ck == 0),
                                stop=(ck == CK - 1),
                            )
            # softmax over n (free dim; logits are O(1), no max-subtraction)
            c_tiles = []
            with nc.named_scope(f"r{r}_sm"):
                for b in range(BL):
                    e_sb = work.tile([128, IC, N], F32, tag="e")
                    nc.scalar.activation(
                        out=e_sb[:], in_=lg_tiles[b][:], func=AF.Exp
                    )
                    s_sb = work.tile([128, IC], F32, tag="s")
                    nc.vector.reduce_sum(
                        out=s_sb[:], in_=e_sb[:], axis=mybir.AxisListType.X
                    )
                    sr_sb = work.tile([128, IC], F32, tag="sr")
                    nc.vector.reciprocal(out=sr_sb[:], in_=s_sb[:])
                    c_bf = work.tile([128, IC, N], BF16, tag="c")
                    c_tiles.append(c_bf)
                    (nc.vector if b % 2 else nc.gpsimd).tensor_tensor(
                        c_bf[:],
                        e_sb[:],
                        sr_sb[:, :, None].to_broadcast([128, IC, N]),
                        MULT,
                    )
            # tt[b][c, n] = sum_i u[i, c] c[i, n]   (T transposed, directly)
            with nc.named_scope(f"r{r}_t"):
                tt_bf = work.tile([128, CK, N, BL], BF16, tag="tt")
                for b in range(BL):
                    tt_ps = ps_tt.tile([128, CK, N], F32, tag="tt")
                    for ck in range(CK):
                        for ic in range(IC):
                            nc.tensor.matmul(
                                out=tt_ps[:, ck, :],
                                lhsT=u_bf[:, b, ic, ck * 128:(ck + 1) * 128],
                                rhs=c_tiles[b][:, ic, :],
                                start=(ic == 0),
                                stop=(ic == IC - 1),
                            )
                    _copy(b, tt_bf[:, :, :, b], tt_ps[:])

        # ---------- pre: paired-capsule stationaries, quadrant extraction ----
        # r == 0: rhs = st (uniform c), both row-halves valid for rhs cols 0:4.
        # r >= 1: rhs = [tt_n | tt_{n+1}] (8 cols); quads [0:64,0:4]/[64:128,4:8].
        with nc.named_scope(f"r{r}_pre"):
            fr = BL if r == 0 else 2 * BL
            pre_ps = ps_pre.tile([128, NP, 2 * BL], F32, tag="pre")
            for k in range(NP):
                for ck in range(CK):
                    rhs = (
                        st_bf[:, ck, :] if r == 0
                        else tt_bf[:, ck, 2 * k:2 * k + 2, :]
                    )
                    nc.tensor.matmul(
                        out=pre_ps[:, k, 0:fr],
                        lhsT=w_bf[:, ck, k * 128:(k + 1) * 128],
                        rhs=rhs,
                        start=(ck == 0),
                        stop=(ck == CK - 1),
                    )
            pre_sb = work.tile([64, NB], F32, tag="pre_sb")
            pre_v = pre_sb[:].rearrange("d (k x) -> d k x", x=2 * BL)
            nc.scalar.copy(
                out=pre_v[:, :, 0:BL], in_=pre_ps[0:64, :, 0:BL]
            )
            nc.vector.tensor_copy(
                out=pre_v[:, :, BL:2 * BL],
                in_=pre_ps[64:128, :, (0 if r == 0 else BL):fr],
            )

        # ---------- squash over d (ones-matmul reduction; rsqrt via bit-trick
        # seed + Newton on DVE keeps the ACT table pinned on exp) ----------
        with nc.named_scope(f"r{r}_sq"):
            sq_sb = work.tile([64, NB], F32, tag="sq")
            nc.vector.tensor_mul(_r(sq_sb[:]), pre_sb[:], pre_sb[:])
            ss_ps = ps.tile([1, NB], F32, tag="sqps")
            nc.tensor.matmul(
                out=ss_ps[:], lhsT=_r(ones_col[0:64, :]), rhs=_r(sq_sb[:]),
                start=True, stop=True,
            )
            # x = sum/N^2 + eps for r==0 (squash of pre/N), else sum + eps
            x_sb = work.tile([1, NB], F32, tag="x")
            nc.vector.tensor_scalar(
                out=x_sb[:], in0=ss_ps[:],
                scalar1=(1.0 / (N * N) if r == 0 else 1.0), scalar2=EPS,
                op0=MULT, op1=mybir.AluOpType.add,
            )
            # y0 = bitcast(0x5f3759df - (bitcast(x) >> 1))
            yb_sb = work.tile([1, NB], U32, tag="yb")
            nc.vector.tensor_scalar(
                out=yb_sb[:], in0=x_sb[:].bitcast(U32), scalar1=1, scalar2=None,
                op0=mybir.AluOpType.logical_shift_right,
            )
            y_sb = work.tile([1, NB], F32, tag="y")
            nc.vector.tensor_tensor(
                y_sb[:].bitcast(U32), magic[:], yb_sb[:],
                mybir.AluOpType.subtract,
            )
            # Newton: y <- y * (1.5 - 0.5 x y^2); 1 step mid-routing (the
            # magnitude error only perturbs the next round's logit scale by
            # ~0.2%), 2 steps for the returned round (rsqrt rel err ~4e-6)
            for it in range(1 if r < ROUTINGS - 1 else 2):
                t1 = work.tile([1, NB], F32, tag="nt1")
                nc.vector.tensor_mul(t1[:], y_sb[:], y_sb[:])
                nc.vector.scalar_tensor_tensor(
                    out=t1[:], in0=t1[:], scalar=-0.5, in1=x_sb[:],
                    op0=MULT, op1=MULT,
                )
                y2 = work.tile([1, NB], F32, tag="y")
                nc.vector.scalar_tensor_tensor(
                    out=_r(y2[:]), in0=t1[:], scalar=1.5, in1=y_sb[:],
                    op0=mybir.AluOpType.add, op1=MULT,
                )
                y_sb = y2
            if r == 0:
                nc.vector.tensor_scalar_mul(_r(y_sb[:]), y_sb[:], 1.0 / N)
            rnb_ps = ps.tile([64, NB], F32, tag="rnb")
            nc.tensor.matmul(
                out=rnb_ps[:], lhsT=_r(ones_row[0:1, 0:64]), rhs=_r(y_sb[:]),
                start=True, stop=True,
            )
            if r < ROUTINGS - 1:
                o_sb = work.tile([64, NB], BF16, tag="o_bf")
                nc.vector.tensor_tensor(o_sb[:], pre_sb[:], rnb_ps[:], MULT)
            else:
                o_sb = work.tile([64, NB], F32, tag="o")
                nc.vector.tensor_tensor(_r(o_sb[:]), pre_sb[:], rnb_ps[:], MULT)

    # ---------- write out: out[b, n, d] = o[d, (n,b)] ----------
    with nc.named_scope("out"):
        ot_ps = ps.tile([128, 64], F32, tag="sqps")
        nc.tensor.transpose(ot_ps[:], o_sb[:], ident[0:64, 0:64])
        ot_sb = work.tile([128, 64], F32, tag="ot")
        nc.scalar.copy(out=ot_sb[:], in_=ot_ps[:])
        out_nbd = bass.AP(
            tensor=out_ap.tensor,
            offset=out_ap.offset,
            ap=[[D, N], [N * D, BL], [1, D]],
        )
        nc.sync.dma_start(out=out_nbd, in_=ot_sb[:])


def build_program():
    nc = bacc.Bacc("TRN2", target_bir_lowering=False, debug=False)
    u_ap = nc.dram_tensor("u", [BL, I, C], F32, kind="ExternalInput").ap()
    w_ap = nc.dram_tensor("w", [C, ND], F32, kind="ExternalInput").ap()
    out_ap = nc.dram_tensor("out", [BL, N, D], F32, kind="ExternalOutput").ap()
    with tile.TileContext(nc) as tc:
        with ExitStack() as ctx:
            _capsule_body(ctx, tc, out_ap, u_ap, w_ap)
    nc.compile()
    return nc


_NC = None


def kernel(u_vecs: np.ndarray, W: np.ndarray) -> np.ndarray:
    global _NC
    u = np.ascontiguousarray(np.asarray(u_vecs, dtype=np.float32))
    w = np.ascontiguousarray(np.asarray(W, dtype=np.float32))
    assert u.shape == (B, I, C) and w.shape == (C, ND)
    if _NC is None:
        _NC = build_program()
    in_maps = [
        {"u": u[i * BL:(i + 1) * BL], "w": w} for i in range(NCORES)
    ]
    res = run_bass_kernel_spmd(_NC, in_maps, list(range(NCORES)))
    return np.concatenate(
        [res.results[i]["out"] for i in range(NCORES)], axis=0
    )


# revision 23
# speedup vs baseline: 1.4760x; 1.2324x over previous
"""Trainium2 Bass kernel for capsule routing (nn_Capsule).

Reference computation:
    u_hat = einsum('bic,ce->bie', u_vecs, W).reshape(B, I, N, D).transpose(0,2,1,3)
    b = 0
    for r in range(3):
        c = softmax(b, axis=1)                      # over capsules n
        out = squash(einsum('bni,bnid->bnd', c, u_hat))
        if r < 2: b = einsum('bnd,bnid->bni', out, u_hat)
    return out    # (B, N, D)

Algebraic restructuring (u_hat never materialized; all uses factor through
u_vecs and W):
    round 0:  c uniform = 1/N  ->  out0 = squash((1/N) * W^T (sum_i u[b,i,:]))
    logits[b,i,n] = sum_c u[b,i,c] V[b,c,n],   V[b,c,n] = sum_d W[c,(n,d)] o[b,n,d]
    tt[b,c,n]    = sum_i softmax(logits)[b,i,n] * u[b,i,c]     (T transposed)
    pre[b,n,d]   = sum_c tt[b,c,n] W[c,(n,d)]   -> out = squash(pre)

PE-efficiency notes (LDWEIGHTS cost ~ stationary_columns/1.2 ns, halved by
fast-weight-load which triggers on 128-column non-fp32 stationaries):
  - every routing matmul uses a bf16 128-column stationary (FWL): u chunks for
    tt, u^T chunks for logits, W[:, 128-col] for pre, paired W^T for V,
  - tt is computed directly in [c, n] layout (u chunk stationary, softmax
    weights streamed) so the per-batch T transposes of the naive layout vanish,
  - pre packs capsule pairs into one [c,128] stationary: out quadrants
    [0:64, 0:4] / [64:128, 4:8] hold pre_n / pre_{n+1}; cross-quadrants are
    discarded.  W^T for V is built with the same pairing (one [c,128]
    transpose per capsule pair),
  - squash rsqrt = bit-trick seed + Newton on DVE, keeping the Scalar
    activation table pinned on exp,
  - input DMAs are 6 x 1MB split across the two HW-DGE rings (SP + ACT).

Sharding: data-parallel over batch, 4 batches per core x 8 cores, W replicated.
"""

import numpy as np
from contextlib import ExitStack

import concourse.bass as bass
import concourse.bacc as bacc
import concourse.tile as tile
from concourse import mybir
from concourse.bass_utils import run_bass_kernel_spmd
from concourse.masks import make_identity

B, I, C = 32, 1024, 256
N, D = 32, 64
ND = N * D
ROUTINGS = 3
EPS = 1e-7
NCORES = 8
BL = B // NCORES  # batches per core
IC = I // 128     # i chunks of 128
CK = C // 128     # c chunks of 128
NB = N * BL       # 128 = (n, b) composite
NP = N // 2       # capsule pairs
NDK = ND // 128   # 128-col chunks of the (n,d) axis == NP
F32 = mybir.dt.float32
F32R = mybir.dt.float32r
U32 = mybir.dt.uint32
BF16 = mybir.dt.bfloat16
MULT = mybir.AluOpType.mult
AF = mybir.ActivationFunctionType
RSQRT_MAGIC = 0x5F3759DF


def _r(ap):
    """View an fp32 AP as float32r: single-pass (FP22) PE matmul instead of
    the 2-pass fp32 split walrus emits otherwise."""
    return ap.bitcast(F32R)


def _capsule_body(ctx: ExitStack, tc: tile.TileContext, out_ap, u_ap, w_ap):
    nc = tc.nc

    ctx.enter_context(nc.allow_low_precision(reason="bf16/fp32r matmul path"))

    const = ctx.enter_context(tc.tile_pool(name="const", bufs=1))
    persist = ctx.enter_context(tc.tile_pool(name="persist", bufs=1))
    work = ctx.enter_context(tc.tile_pool(name="work", bufs=4))

    # ---- constants ----
    ident = const.tile([128, 128], F32)
    make_identity(nc, ident[:])
    ident_bf = const.tile([128, 128], BF16)
    make_identity(nc, ident_bf[:])
    ones_f = const.tile([128, 2], F32)
    nc.gpsimd.memset(ones_f[:], 1.0)
    ones_bf = const.tile([128, 1], BF16)
    nc.gpsimd.memset(ones_bf[:], 1.0)
    ones_col = const.tile([128, 1], F32)
    nc.vector.tensor_copy(out=_r(ones_col[:]), in_=ones_f[:, 0:1])
    ones_row = const.tile([1, 128], F32)
    nc.vector.tensor_copy(out=_r(ones_row[:]), in_=ones_f[0:1, 0:1].to_broadcast([1, 128]))
    magic = const.tile([1, NB], U32)
    nc.gpsimd.memset(magic[:], RSQRT_MAGIC)

    # ---- persistent SBUF tensors ----
    w_sb = persist.tile([128, CK, ND], F32)       # [c, ck, (n,d)]
    w_bf = persist.tile([128, CK, ND], BF16)      # bf16 copy (pre stationaries)
    wt_ev = persist.tile([64, NP, C], BF16)       # [d, p, c] = W_2p^T (V stationaries)
    wt_od = persist.tile([64, NP, C], BF16)       # [d, p, c] = W_{2p+1}^T
    u_sb = persist.tile([128, BL, IC, C], F32)    # [i, b, ic, c]
    u_bf = persist.tile([128, BL, IC, C], BF16)   # bf16 copy (tt stationaries)
    ut_bf = persist.tile([128, BL, CK, I], BF16)  # [c, b, ck, i] (lg stationaries)
    st_bf = persist.tile([128, CK, BL], BF16)     # [c, ck, b] column sums of u

    # ---- input DMAs: 6 x 1MB, split across the SP and ACT HW-DGE rings ----
    # sync ring: u[b=0], u[b=1]; scalar ring: W, u[b=2], u[b=3]
    def dma_u(eng, b):
        src = bass.AP(
            tensor=u_ap.tensor,
            offset=u_ap.offset + b * I * C,
            ap=[[C, 128], [128 * C, IC], [1, C]],
        )
        eng.dma_start(out=_r(u_sb[:, b, :, :]), in_=_r(src))

    # All DMAs on the SP ring (ACT-issued DMAs wedge the device under this
    # runtime); W interleaved late since its consumers run later
    def dma_w(ck):
        nc.sync.dma_start(
            out=_r(w_sb[:, ck, :]), in_=_r(w_ap[ck * 128:(ck + 1) * 128, :])
        )

    dma_u(nc.sync, 0)
    dma_u(nc.sync, 1)
    dma_u(nc.sync, 2)
    dma_w(0)
    dma_u(nc.sync, 3)
    dma_w(1)

    # ---- setup: casts, transposes, column sums ----
    def _copy(idx, out, in_):
        # PSUM -> SBUF evacuation: only ACT and DVE can read PSUM (and both
        # cast ~3x faster than GpSimd, which is kept off the big-tensor path)
        if idx % 2 == 0:
            nc.scalar.copy(out=out, in_=in_)
        else:
            nc.vector.tensor_copy(out=out, in_=in_)

    with tc.tile_pool(name="ps_su", bufs=3, space="PSUM") as ps_su, \
            tc.tile_pool(name="ps_sw", bufs=2, space="PSUM") as ps_sw, \
            tc.tile_pool(name="ps_st", bufs=1, space="PSUM") as ps_st, \
            nc.named_scope("setup"):
        # W cast to bf16 (ACT; DVE is busy with the u casts)
        for ck in range(CK):
            nc.scalar.copy(out=w_bf[:, ck, :], in_=w_sb[:, ck, :])
        st_ps = ps_st.tile([128, CK, BL], F32, tag="st")
        for b in range(BL):
            # cast u[b] to bf16 in halves so transposes chase the DMA
            for h in range(2):
                _copy(
                    b + h,
                    u_bf[:, b, h * 4:(h + 1) * 4, :],
                    u_sb[:, b, h * 4:(h + 1) * 4, :],
                )
            # column sums st[c, b] = sum_i u[i, c] as ones-matmuls (off the
            # DVE, and not dependent on the transposes' PSUM evacuation)
            for ck in range(CK):
                for ic in range(IC):
                    nc.tensor.matmul(
                        out=st_ps[:, ck, b:b + 1],
                        lhsT=u_bf[:, b, ic, ck * 128:(ck + 1) * 128],
                        rhs=ones_bf[:],
                        start=(ic == 0),
                        stop=(ic == IC - 1),
                    )
            # transpose 4-chunk groups: ut[c, b, ck, :]
            for ck in range(CK):
                for j in range(IC // 4):
                    ut_ps = ps_su.tile([128, 4, 128], BF16, tag="ut")
                    for t in range(4):
                        icx = 4 * j + t
                        nc.tensor.transpose(
                            ut_ps[:, t, :],
                            u_bf[:, b, icx, ck * 128:(ck + 1) * 128],
                            ident_bf[:],
                        )
                    _copy(
                        b * 4 + ck * 2 + j,
                        ut_bf[:, b, ck, j * 512:(j + 1) * 512],
                        ut_ps[:].rearrange("c t i -> c (t i)"),
                    )
            # interleave W transposes behind the first u batch
            if b == 0:
                for p in range(NP):
                    wt_ps = ps_sw.tile([128, CK, 128], BF16, tag="wt")
                    for ck in range(CK):
                        nc.tensor.transpose(
                            wt_ps[:, ck, :],
                            w_bf[:, ck, p * 128:(p + 1) * 128],
                            ident_bf[:],
                        )
                    _copy(
                        p,
                        wt_ev[:, p, :],
                        wt_ps[0:64, :, :].rearrange("d k c -> d (k c)"),
                    )
                    _copy(
                        p + 1,
                        wt_od[:, p, :],
                        wt_ps[64:128, :, :].rearrange("d k c -> d (k c)"),
                    )
        nc.vector.tensor_copy(out=st_bf[:], in_=st_ps[:])

    ps = ctx.enter_context(tc.tile_pool(name="ps_main", bufs=1, space="PSUM"))
    ps_lg = ctx.enter_context(tc.tile_pool(name="ps_lg", bufs=3, space="PSUM"))
    ps_tt = ctx.enter_context(tc.tile_pool(name="ps_tt", bufs=1, space="PSUM"))
    ps_pre = ctx.enter_context(tc.tile_pool(name="ps_pre", bufs=1, space="PSUM"))

    o_sb = None
    for r in range(ROUTINGS):
        tt_bf = None
        if r > 0:
            # V[b][c, n] = sum_d W[c,(n,d)] o[b,n,d]; stationary = paired W^T
            with nc.named_scope(f"r{r}_v"):
                v_ps = ps.tile([128, CK, N, BL], F32, tag="v")
                for ck in range(CK):
                    for n in range(N):
                        wt = wt_ev if n % 2 == 0 else wt_od
                        nc.tensor.matmul(
                            out=v_ps[:, ck, n, :],
                            lhsT=wt[:, n // 2, ck * 128:(ck + 1) * 128],
                            rhs=o_sb[:, n * BL:(n + 1) * BL],
                            start=True,
                            stop=True,
                        )
                v_bf = work.tile([128, CK, N, BL], BF16, tag="v_bf")
                nc.scalar.copy(out=v_bf[:], in_=v_ps[:])

            # logits[b][i, n] = sum_c ut[c, i] V[c, n]   (all b first: the
            # per-b softmax chains run behind the PE's logits stream)
            lg_tiles = []
            with nc.named_scope(f"r{r}_lg"):
                for b in range(BL):
                    lg_ps = ps_lg.tile([128, IC, N], F32, tag="lg")
                    lg_tiles.append(lg_ps)
                    for ic in range(IC):
                        for ck in range(CK):
                            nc.tensor.matmul(
                                out=lg_ps[:, ic, :],
                                lhsT=ut_bf[:, b, ck, ic * 128:(ic + 1) * 128],
                                rhs=v_bf[:, ck, :, b],
                                start=(ck == 0),
                                stop=(ck == CK - 1),
                            )
            # softmax over n (free dim; logits are O(1), no max-subtraction)
            c_tiles = []
            with nc.named_scope(f"r{r}_sm"):
                for b in range(BL):
                    e_sb = work.tile([128, IC, N], F32, tag="e")
                    nc.scalar.activation(
                        out=e_sb[:], in_=lg_tiles[b][:], func=AF.Exp
                    )
                    s_sb = work.tile([128, IC], F32, tag="s")
                    nc.vector.reduce_sum(
                        out=s_sb[:], in_=e_sb[:], axis=mybir.AxisListType.X
                    )
                    sr_sb = work.tile([128, IC], F32, tag="sr")
                    nc.vector.reciprocal(out=sr_sb[:], in_=s_sb[:])
                    c_bf = work.tile([128, IC, N], BF16, tag="c")
                    c_tiles.append(c_bf)
                    nc.gpsimd.tensor_tensor(
                        c_bf[:],
                        e_sb[:],
                        sr_sb[:, :, None].to_broadcast([128, IC, N]),
                        MULT,
                    )
            # tt[b][c, n] = sum_i u[i, c] c[i, n]   (T transposed, directly)
            with nc.named_scope(f"r{r}_t"):
                tt_bf = work.tile([128, CK, N, BL], BF16, tag="tt")
                for b in range(BL):
                    tt_ps = ps_tt.tile([128, CK, N], F32, tag="tt")
                    for ck in range(CK):
                        for ic in range(IC):
                            nc.tensor.matmul(
                                out=tt_ps[:, ck, :],
                                lhsT=u_bf[:, b, ic, ck * 128:(ck + 1) * 128],
                                rhs=c_tiles[b][:, ic, :],
                                start=(ic == 0),
                                stop=(ic == IC - 1),
                            )
                    _copy(b, tt_bf[:, :, :, b], tt_ps[:])

        # ---------- pre: paired-capsule stationaries, quadrant extraction ----
        # r == 0: rhs = st (uniform c), both row-halves valid for rhs cols 0:4.
        # r >= 1: rhs = [tt_n | tt_{n+1}] (8 cols); quads [0:64,0:4]/[64:128,4:8].
        with nc.named_scope(f"r{r}_pre"):
            fr = BL if r == 0 else 2 * BL
            pre_ps = ps_pre.tile([128, NP, 2 * BL], F32, tag="pre")
            for k in range(NP):
                for ck in range(CK):
                    rhs = (
                        st_bf[:, ck, :] if r == 0
                        else tt_bf[:, ck, 2 * k:2 * k + 2, :]
                    )
                    nc.tensor.matmul(
                        out=pre_ps[:, k, 0:fr],
                        lhsT=w_bf[:, ck, k * 128:(k + 1) * 128],
                        rhs=rhs,
                        start=(ck == 0),
                        stop=(ck == CK - 1),
                    )
            pre_sb = work.tile([64, NB], F32, tag="pre_sb")
            pre_v = pre_sb[:].rearrange("d (k x) -> d k x", x=2 * BL)
            nc.scalar.copy(
                out=pre_v[:, :, 0:BL], in_=pre_ps[0:64, :, 0:BL]
            )
            nc.vector.tensor_copy(
                out=pre_v[:, :, BL:2 * BL],
                in_=pre_ps[64:128, :, (0 if r == 0 else BL):fr],
            )

        # ---------- squash over d (ones-matmul reduction; rsqrt via bit-trick
        # seed + Newton on DVE keeps the ACT table pinned on exp) ----------
        with nc.named_scope(f"r{r}_sq"):
            sq_sb = work.tile([64, NB], F32, tag="sq")
            nc.vector.tensor_mul(_r(sq_sb[:]), pre_sb[:], pre_sb[:])
            ss_ps = ps.tile([1, NB], F32, tag="sqps")
            nc.tensor.matmul(
                out=ss_ps[:], lhsT=_r(ones_col[0:64, :]), rhs=_r(sq_sb[:]),
                start=True, stop=True,
            )
            # x = sum/N^2 + eps for r==0 (squash of pre/N), else sum + eps
            x_sb = work.tile([1, NB], F32, tag="x")
            nc.vector.tensor_scalar(
                out=x_sb[:], in0=ss_ps[:],
                scalar1=(1.0 / (N * N) if r == 0 else 1.0), scalar2=EPS,
                op0=MULT, op1=mybir.AluOpType.add,
            )
            # y0 = bitcast(0x5f3759df - (bitcast(x) >> 1))
            yb_sb = work.tile([1, NB], U32, tag="yb")
            nc.vector.tensor_scalar(
                out=yb_sb[:], in0=x_sb[:].bitcast(U32), scalar1=1, scalar2=None,
                op0=mybir.AluOpType.logical_shift_right,
            )
            y_sb = work.tile([1, NB], F32, tag="y")
            nc.vector.tensor_tensor(
                y_sb[:].bitcast(U32), magic[:], yb_sb[:],
                mybir.AluOpType.subtract,
            )
            # Newton: y <- y * (1.5 - 0.5 x y^2); 1 step mid-routing (the
            # magnitude error only perturbs the next round's logit scale by
            # ~0.2%), 2 steps for the returned round (rsqrt rel err ~4e-6)
            for it in range(1 if r < ROUTINGS - 1 else 2):
                t1 = work.tile([1, NB], F32, tag="nt1")
                nc.vector.tensor_mul(t1[:], y_sb[:], y_sb[:])
                nc.vector.scalar_tensor_tensor(
                    out=t1[:], in0=t1[:], scalar=-0.5, in1=x_sb[:],
                    op0=MULT, op1=MULT,
                )
                y2 = work.tile([1, NB], F32, tag="y")
                nc.vector.scalar_tensor_tensor(
                    out=_r(y2[:]), in0=t1[:], scalar=1.5, in1=y_sb[:],
                    op0=mybir.AluOpType.add, op1=MULT,
                )
                y_sb = y2
            if r == 0:
                nc.vector.tensor_scalar_mul(_r(y_sb[:]), y_sb[:], 1.0 / N)
            rnb_ps = ps.tile([64, NB], F32, tag="rnb")
            nc.tensor.matmul(
                out=rnb_ps[:], lhsT=_r(ones_row[0:1, 0:64]), rhs=_r(y_sb[:]),
                start=True, stop=True,
            )
            if r < ROUTINGS - 1:
                o_sb = work.tile([64, NB], BF16, tag="o_bf")
                nc.vector.tensor_tensor(o_sb[:], pre_sb[:], rnb_ps[:], MULT)
            else:
                o_sb = work.tile([64, NB], F32, tag="o")
                nc.vector.tensor_tensor(_r(o_sb[:]), pre_sb[:], rnb_ps[:], MULT)

    # ---------- write out: out[b, n, d] = o[d, (n,b)] ----------
    with nc.named_scope("out"):
        ot_ps = ps.tile([128, 64], F32, tag="sqps")
        nc.tensor.transpose(ot_ps[:], o_sb[:], ident[0:64, 0:64])
        ot_sb = work.tile([128, 64], F32, tag="ot")
        nc.scalar.copy(out=ot_sb[:], in_=ot_ps[:])
        out_nbd = bass.AP(
            tensor=out_ap.tensor,
            offset=out_ap.offset,
            ap=[[D, N], [N * D, BL], [1, D]],
        )
        nc.sync.dma_start(out=out_nbd, in_=ot_sb[:])


def build_program():
    nc = bacc.Bacc("TRN2", target_bir_lowering=False, debug=False)
    u_ap = nc.dram_tensor("u", [BL, I, C], F32, kind="ExternalInput").ap()
    w_ap = nc.dram_tensor("w", [C, ND], F32, kind="ExternalInput").ap()
    out_ap = nc.dram_tensor("out", [BL, N, D], F32, kind="ExternalOutput").ap()
    with tile.TileContext(nc) as tc:
        with ExitStack() as ctx:
            _capsule_body(ctx, tc, out_ap, u_ap, w_ap)
    nc.compile()
    return nc


_NC = None


def kernel(u_vecs: np.ndarray, W: np.ndarray) -> np.ndarray:
    global _NC
    u = np.ascontiguousarray(np.asarray(u_vecs, dtype=np.float32))
    w = np.ascontiguousarray(np.asarray(W, dtype=np.float32))
    assert u.shape == (B, I, C) and w.shape == (C, ND)
    if _NC is None:
        _NC = build_program()
    in_maps = [
        {"u": u[i * BL:(i + 1) * BL], "w": w} for i in range(NCORES)
    ]
    res = run_bass_kernel_spmd(_NC, in_maps, list(range(NCORES)))
    return np.concatenate(
        [res.results[i]["out"] for i in range(NCORES)], axis=0
    )
